# revision 35
# baseline (speedup 1.0000x reference)
"""MiTA sparse attention kernel for Trainium2 (8 NeuronCores, Bass/Tile).

Sharding: data-parallel over batch B=16 -> 2 batches per core; all 12 heads
of a batch are processed on the same core.

Math (per batch b, head h; d=64, M=25 experts, kv_topk=12, router_topk=2):
  qkv = x @ Wqkv ; router = AdaptiveAvgPool(q-grid)
  rak = router k^T ; kidx = top12(rak) ; gate = q router^T ; top2 experts/query
  single softmax over {agent logits (25)} U {selected experts' top12 keys}
  out = (e_a @ (softmax(rak*s) @ v) + e_m @ v[kidx]) / denom ; proj.

Implementation notes (v2):
  - selection chain reassociated to avoid any fp32 q/k projection:
      router = pool(x) @ Wq          (pooling exact, matmul fp32)
      rak_h  = (router_h Wk_h^T) x^T (fp32; tiny [25,768] intermediate A_h)
      gate_h = x (router_h Wq_h^T)^T (fp32; tiny B_h)
    measured 0/4800 kidx and 0/110784 eidx top-k flips vs the reference
    ordering (perturbation ~1e-7 vs bf16's 4e-3 which flips 283 rows).
  - value path all bf16: q/k tiles from bf16 matmuls, full-577-key moba
    attention weighted by the multiplicity mask W = sel@mask12 in {0,1,2}.
  - rak/mask/sel tiles stack 4 heads at 32-partition strides so the K=32
    W^T matmuls and K=64 qm matmuls issue as adjacent row-group pairs
    (distinct PSUM banks) for PE-array tile concurrency.
  - proj contracts head PAIRS in single K=128 matmuls (outT2 tiles hold
    head 2g at partitions 0:64 and 2g+1 at 64:128 via DMA shift).
  - reciprocals batched per 4-head group ([4,577] once vs [1,577] x4).
  - W mask PSUM->SBUF casts on gpsimd; EW multiply runs bf16*bf16 (DVE 2x).
  - softmax unstabilized (logit scale ~0.3) as in the baseline; denominators
    from ones-augmented value matrices.
"""

import sys

for _p in ("/opt/trn_rl_repo",):
    if _p not in sys.path:
        sys.path.insert(0, _p)

from contextlib import ExitStack

import numpy as np
import ml_dtypes

import concourse.bacc as bacc
import concourse.tile as tile
import concourse.mybir as mybir
from concourse.bass_utils import run_bass_kernel_spmd
from concourse.masks import make_identity

FP32 = mybir.dt.float32
BF16 = mybir.dt.bfloat16
ALU = mybir.AluOpType
ACTF = mybir.ActivationFunctionType
AX = mybir.AxisListType

B, N, C = 16, 577, 768
H, D, M, POOL = 12, 64, 25, 5
NB = 2  # batches per core
NCORES = 8
SCALE = float(D) ** -0.5  # 0.125
NEGBIG = -1e30
NTS = [(i * 128, min(128, N - i * 128)) for i in range((N + 127) // 128)]  # 5
CTS = 6  # 128-col tiles per 768
import os
PH = int(os.environ.get("MITA_PH", "9"))
DBG = int(os.environ.get("MITA_DBG", "0"))

# adaptive-pool regions of the 24x24 grid: (y0, ny, x0, nx, 1/(ny*nx))
_BINS = [(int(np.floor(i * 24 / POOL)), int(np.ceil((i + 1) * 24 / POOL)))
         for i in range(POOL)]
REGIONS = []
for _r in range(POOL):
    for _c in range(POOL):
        _y0, _y1 = _BINS[_r]
        _x0, _x1 = _BINS[_c]
        REGIONS.append((_y0, _y1 - _y0, _x0, _x1 - _x0,
                        1.0 / ((_y1 - _y0) * (_x1 - _x0))))


def _divide_head(nc, p_w, g4, s, den4, drow, numT, outT2):
    h = g4 * 4 + s
    den1 = p_w.tile([1, N], FP32, tag="den1", bufs=2, name="den1")
    nc.sync.dma_start(den1[:, :], den4[drow:drow + 1, :])
    rb = p_w.tile([64, N], FP32, tag="rb", bufs=1, name="rb")
    nc.gpsimd.partition_broadcast(rb[:, :], den1[0:1, :], channels=64)
    if h % 2 == 0:
        nc.vector.tensor_tensor(outT2[h // 2][0:64, :], numT[0:64, :],
                                rb[:, :], op=ALU.mult)
    else:
        oth = p_w.tile([64, N], BF16, tag="oth", bufs=2, name="oth")
        nc.vector.tensor_tensor(oth[:, :], numT[0:64, :], rb[:, :],
                                op=ALU.mult)
        nc.sync.dma_start(outT2[h // 2][64:128, :], oth[:, :])


def _emit(tc, io):
    nc = tc.nc
    ctx = tc._ctx

    p_const = ctx.enter_context(tc.tile_pool(name="const", bufs=1))
    p_w = ctx.enter_context(tc.tile_pool(name="work", bufs=1))
    p_ew = ctx.enter_context(tc.tile_pool(name="ew", bufs=8))
    # PSUM pools: 3 + 3 + 2 = 8 banks.
    ps_a = ctx.enter_context(tc.tile_pool(name="ps_a", bufs=3, space="PSUM"))
    ps_w = ctx.enter_context(tc.tile_pool(name="ps_w", bufs=3, space="PSUM"))
    ps_v = ctx.enter_context(tc.tile_pool(name="ps_v", bufs=2, space="PSUM"))

    # ---- constants / weights ----
    ident_bf = p_const.tile([128, 128], BF16, tag="idbf")
    make_identity(nc, ident_bf[:])
    ident32 = p_const.tile([128, 128], FP32, tag="id32")
    make_identity(nc, ident32[:])
    ones_bf = p_const.tile([1, 128], BF16, tag="ones")
    nc.vector.memset(ones_bf[:], 1.0)
    wpool = p_const.tile([128, M], FP32, tag="wpool")
    for m, (_, _, _, _, wgt) in enumerate(REGIONS):
        nc.vector.memset(wpool[:, m:m + 1], wgt)

    wq32 = []  # q-columns of Wqkv, fp32 [128, 768] x6
    for kc in range(CTS):
        t = p_const.tile([128, C], FP32, tag=f"wq32_{kc}", name=f"wq32_{kc}")
        nc.sync.dma_start(t[:], io["wq32"][kc * 128:(kc + 1) * 128, :])
        wq32.append(t)
    wqk_bf = []  # q,k columns bf16 [128, 1536] x6
    for kc in range(CTS):
        t = p_const.tile([128, 2 * C], BF16, tag=f"wqkb{kc}", name=f"wqkb{kc}")
        nc.sync.dma_start(t[:], io["wqk_bf"][kc * 128:(kc + 1) * 128, :])
        wqk_bf.append(t)
    wv_sb = []
    for kc in range(CTS):
        t = p_const.tile([128, C], BF16, tag=f"wv{kc}", name=f"wv{kc}")
        nc.sync.dma_start(t[:], io["wv"][kc * 128:(kc + 1) * 128, :])
        wv_sb.append(t)
    wproj2 = []  # head-pair tiles [128, 768] bf16 (rows g*128..)
    for g in range(6):
        t = p_const.tile([128, C], BF16, tag=f"wp{g}", name=f"wp{g}")
        nc.sync.dma_start(t[:], io["wproj"][g * 128:(g + 1) * 128, :])
        wproj2.append(t)
    bproj_sb = p_const.tile([1, C], BF16, tag="bproj")
    nc.sync.dma_start(bproj_sb[:], io["bproj"][:, :])
    bb128 = p_const.tile([128, C], BF16, tag="bb128")
    nc.gpsimd.partition_broadcast(bb128[:, :], bproj_sb[0:1, :], channels=128)

    for b in range(NB):
        # ---- load x^T (bf16 first: unblocks v/qk projections) ----
        xTbf = []
        for kc in range(CTS):
            t = p_w.tile([128, N], BF16, tag=f"t{kc}", name=f"xTbf_{kc}")
            nc.sync.dma_start(t[:], io["xT_bf16"][b, kc * 128:(kc + 1) * 128, :])
            xTbf.append(t)
        xT32 = []
        for kc in range(CTS):
            t = p_w.tile([128, N], FP32, tag=f"w{kc}", name=f"xT32_{kc}")
            nc.sync.dma_start(t[:], io["xT_f32"][b, kc * 128:(kc + 1) * 128, :])
            xT32.append(t)

        # ---- v natural bf16 with ones-augmentation: [n, 12*65] ----
        v_sb = []
        for i, (n0, nsz) in enumerate(NTS):
            pa = ps_a.tile([128, 512], FP32, tag="bank_a")
            pb = ps_a.tile([128, 256], FP32, tag="bank_a")
            for kc in range(CTS):
                nc.tensor.matmul(pa[:nsz, :], xTbf[kc][:, n0:n0 + nsz],
                                 wv_sb[kc][:, 0:512], start=(kc == 0), stop=(kc == 5))
            for kc in range(CTS):
                nc.tensor.matmul(pb[:nsz, :], xTbf[kc][:, n0:n0 + nsz],
                                 wv_sb[kc][:, 512:768], start=(kc == 0), stop=(kc == 5))
            t = p_w.tile([128, H * 65], BF16, tag=f"v{i}", name=f"v_{i}")
            nc.vector.tensor_copy(
                t[:nsz].rearrange("p (h e) -> p h e", e=65)[:, 0:8, 0:64],
                pa[:nsz].rearrange("p (h e) -> p h e", e=64))
            nc.vector.tensor_copy(
                t[:nsz].rearrange("p (h e) -> p h e", e=65)[:, 8:12, 0:64],
                pb[:nsz].rearrange("p (h e) -> p h e", e=64))
            nc.vector.memset(
                t[:nsz].rearrange("p (h e) -> p h e", e=65)[:, :, 64:65], 1.0)
            v_sb.append(t)

        # ---- xp = pool(x-grid): [c-chunk, 25] fp32 x6 ----
        xpT = []
        for ct in range(CTS):
            t = p_w.tile([128, 32], FP32, tag=f"xp{ct}", name=f"xpT_{ct}")
            grid = xT32[ct][:, 0:576].rearrange("p (y x) -> p y x", x=24)
            for m, (y0, ny, x0, nx, _) in enumerate(REGIONS):
                nc.vector.tensor_reduce(
                    t[:, m:m + 1], grid[:, y0:y0 + ny, x0:x0 + nx],
                    axis=AX.XY, op=ALU.add)
            nc.vector.tensor_mul(t[:, 0:M], t[:, 0:M], wpool[:])
            nc.vector.memset(t[:, M:32], 0.0)
            xpT.append(t)

        # ---- RT = router^T [(h,d)-chunk, 25] fp32: RTall [128, 6*32] ----
        rt_ps = ps_w.tile([128, 192], FP32, tag="bank_w", name="rt_ps")
        for ct in range(CTS):
            for kc in range(CTS):
                nc.tensor.matmul(rt_ps[:, ct * 32:(ct + 1) * 32],
                                 wq32[kc][:, ct * 128:(ct + 1) * 128],
                                 xpT[kc][:, :],
                                 start=(kc == 0), stop=(kc == 5))
        RTall = p_w.tile([128, 192], FP32, tag="rtall", name="RTall")
        nc.scalar.copy(RTall[:], rt_ps[:])

        # ---- stream wqkT tiles: BT (q-half) and AT4 (k-half) ----
        # BT[kc] [128, 384] fp32 : gate rhs, head h at cols h*32 (25 used)
        # AT4[g4][ct] [128, 128] fp32 : rak lhsT, head s=h%4 at cols s*32
        BT = [p_w.tile([128, H * 32], FP32, tag=f"bt{kc}", name=f"bt{kc}")
              for kc in range(CTS)]
        AT4 = [[p_w.tile([128, 128], FP32, tag=f"at{g4}_{ct}",
                         name=f"at{g4}_{ct}") for ct in range(CTS)]
               for g4 in range(3)]
        # block-diagonal router tiles: R2[ti] [128, 64] with head-even RT in
        # the top-left [0:64, 0:32] block and head-odd in [64:128, 32:64];
        # zeros elsewhere kill the cross-head terms of a K=128 contraction.
        R2 = []
        for tq in range(CTS):
            t = p_w.tile([128, 64], FP32, tag=f"r2_{tq}", name=f"r2_{tq}")
            nc.vector.memset(t[:], 0.0)
            nc.scalar.copy(t[0:64, 0:32], RTall[0:64, tq * 32:(tq + 1) * 32])
            nc.scalar.copy(t[64:128, 32:64],
                           RTall[64:128, tq * 32:(tq + 1) * 32])
            R2.append(t)
        for ti in range(12):
            wt = p_w.tile([128, C], FP32, tag="wqkt", name="wqkt", bufs=2)
            nc.sync.dma_start(wt[:], io["wqkT"][ti * 128:(ti + 1) * 128, :])
            ab_ps = ps_w.tile([128, 384], FP32, tag="bank_w", name="ab_ps")
            for ct in range(CTS):
                nc.tensor.matmul(
                    ab_ps[:, ct * 64:ct * 64 + 64],
                    wt[:, ct * 128:(ct + 1) * 128],
                    R2[ti % 6][:, :],
                    start=True, stop=True)
            if ti < 6:  # q-half -> BT
                for ct in range(CTS):
                    nc.scalar.copy(BT[ct][:, ti * 64:(ti + 1) * 64],
                                   ab_ps[:, ct * 64:(ct + 1) * 64])
            else:  # k-half -> AT4
                tk = ti - 6
                g4, half = tk // 2, tk % 2
                for ct in range(CTS):
                    nc.scalar.copy(AT4[g4][ct][:, half * 64:half * 64 + 64],
                                   ab_ps[:, ct * 64:(ct + 1) * 64])

        # ---- qk^T bf16 direct: 12 tiles [128, 577] ----
        qkTbf = []
        for ct in range(2 * CTS):
            pa = ps_a.tile([128, 512], FP32, tag="bank_a")
            pb = ps_a.tile([128, 65], FP32, tag="bank_a")
            for kc in range(CTS):
                nc.tensor.matmul(pa[:, :], wqk_bf[kc][:, ct * 128:(ct + 1) * 128],
                                 xTbf[kc][:, 0:512], start=(kc == 0), stop=(kc == 5))
            for kc in range(CTS):
                nc.tensor.matmul(pb[:, :], wqk_bf[kc][:, ct * 128:(ct + 1) * 128],
                                 xTbf[kc][:, 512:577], start=(kc == 0), stop=(kc == 5))
            t = p_w.tile([128, N], BF16, tag=f"qkb{ct}", name=f"qkTbf_{ct}")
            nc.scalar.copy(t[:, 0:512], pa[:])
            nc.scalar.copy(t[:, 512:577], pb[:])
            qkTbf.append(t)

        if PH < 2:
            continue
        # ---- rak4[g4] [128, 577] fp32 (4 heads, 32-stride) ----
        rak4, mask12 = [], []
        for g4 in range(3):
            ra = ps_a.tile([128, 512], FP32, tag="bank_a")
            rb_ = ps_a.tile([128, 65], FP32, tag="bank_a")
            for ct in range(CTS):
                nc.tensor.matmul(ra[:, :], AT4[g4][ct][:, :],
                                 xT32[ct][:, 0:512],
                                 start=(ct == 0), stop=(ct == 5))
            for ct in range(CTS):
                nc.tensor.matmul(rb_[:, :], AT4[g4][ct][:, :],
                                 xT32[ct][:, 512:577],
                                 start=(ct == 0), stop=(ct == 5))
            rk_sb = p_w.tile([128, N], FP32, tag=f"rak{g4}", name=f"rak{g4}")
            nc.scalar.copy(rk_sb[:, 0:512], ra[:])
            nc.scalar.copy(rk_sb[:, 512:577], rb_[:])
            rak4.append(rk_sb)
            # top-12 threshold -> mask12 (pad rows -> all-ones, harmless:
            # the matching selT pad rows are zero)
            r8 = p_w.tile([128, 8], FP32, tag="r8", bufs=2)
            rr = p_w.tile([128, N], FP32, tag="den4", bufs=1, name="rr")
            r8b = p_w.tile([128, 8], FP32, tag="r8b", bufs=2)
            nc.vector.max(out=r8[:], in_=rk_sb[:])
            nc.vector.match_replace(out=rr[:], in_to_replace=r8[:],
                                    in_values=rk_sb[:], imm_value=NEGBIG)
            nc.vector.max(out=r8b[:], in_=rr[:])
            mk = p_w.tile([128, N], BF16, tag=f"mk{g4}", name=f"mk{g4}",
                          bufs=1)
            nc.vector.tensor_scalar(mk[:], rk_sb[:], r8b[:, 3:4], None,
                                    op0=ALU.is_ge)
            mask12.append(mk)

        # ---- PT4[i] [128, 384] bf16 = exp(SCALE * rak^T) ----
        PT4 = [p_w.tile([128, 3 * 128], BF16, tag=f"pt{i}", name=f"PT4_{i}")
               for i in range(5)]
        for g4 in range(3):
            for i, (j0, jsz) in enumerate(NTS):
                tp = ps_v.tile([128, 128], FP32, tag="bank_v", name="tp")
                nc.tensor.matmul(tp[0:jsz, 0:128], rak4[g4][:, j0:j0 + jsz],
                                 ident32[0:128, 0:128], is_transpose=True,
                                 start=True, stop=True, skip_group_check=True)
                nc.scalar.activation(PT4[i][:jsz, g4 * 128:(g4 + 1) * 128],
                                     tp[:jsz, :], ACTF.Exp, scale=SCALE)

        # ---- gate [n, (h,m)] fp32 -> sel/gm -> transposes ----
        selT4 = [p_w.tile([128, N], BF16, tag=f"sT{c}", name=f"selT{c}")
                 for c in range(3)]
        gmT4 = [p_w.tile([128, N], BF16, tag=f"gT{c}", name=f"gmT{c}")
                for c in range(3)]
        for i, (n0, nsz) in enumerate(NTS):
            gp = ps_a.tile([128, H * 32], FP32, tag="bank_a")
            for kc in range(CTS):
                nc.tensor.matmul(gp[:nsz, :], xT32[kc][:, n0:n0 + nsz],
                                 BT[kc][:, :], start=(kc == 0), stop=(kc == 5))
            gate_sb = p_w.tile([128, H * 32], FP32, tag="gate", bufs=2)
            nc.scalar.copy(gate_sb[:nsz, :], gp[:nsz, :])
            nc.vector.memset(
                gate_sb[:nsz].rearrange("p (h e) -> p h e", e=32)[:, :, M:32],
                NEGBIG)
            sel_sb = p_w.tile([128, H * 32], BF16, tag="sel", bufs=2)
            gm_sb = p_w.tile([128, H * 32], BF16, tag="gm", bufs=2)
            m8 = p_w.tile([128, 8], FP32, tag="m8", bufs=2)
            nc.vector.tensor_copy(gm_sb[:nsz, :], gate_sb[:nsz, :])
            for h in range(H):
                seg = slice(h * 32, (h + 1) * 32)
                nc.vector.max(out=m8[:nsz, :], in_=gate_sb[:nsz, seg])
                nc.vector.tensor_scalar(
                    sel_sb[:nsz, seg], gate_sb[:nsz, seg], m8[:nsz, 1:2], None,
                    op0=ALU.is_ge)
            for ch in range(3):
                pt = ps_w.tile([128, 128], BF16, tag="bank_w")
                nc.tensor.matmul(pt[0:128, 0:nsz],
                                 sel_sb[:nsz, ch * 128:(ch + 1) * 128],
                                 ident_bf[0:nsz, 0:nsz],
                                 is_transpose=True, start=True, stop=True,
                                 skip_group_check=True)
                nc.scalar.copy(selT4[ch][:, n0:n0 + nsz], pt[0:128, 0:nsz])
                pt2 = ps_w.tile([128, 128], BF16, tag="bank_w")
                nc.tensor.matmul(pt2[0:128, 0:nsz],
                                 gm_sb[:nsz, ch * 128:(ch + 1) * 128],
                                 ident_bf[0:nsz, 0:nsz],
                                 is_transpose=True, start=True, stop=True,
                                 skip_group_check=True)
                nc.scalar.copy(gmT4[ch][:, n0:n0 + nsz], pt2[0:128, 0:nsz])

        # ---- eah: per-head e_a^T at partition base 0 (DMA shift + exp) ----
        eah = []
        for h in range(H):
            s32 = (h % 4) * 32
            gmh = p_w.tile([32, N], BF16, tag="gmh", bufs=2, name="gmh")
            nc.sync.dma_start(gmh[:], gmT4[h // 4][s32:s32 + 32, :])
            t = p_w.tile([32, N], BF16, tag=f"eah{h % 6}", bufs=2,
                         name=f"eah{h}")
            nc.scalar.activation(t[:], gmh[:], ACTF.Exp, scale=SCALE)
            eah.append(t)

        if PH < 3:
            continue
        # ---- av4[g4] [128, 65]: agent values, 4 heads col-grouped ----
        avh = []
        for g4 in range(3):
            au = ps_v.tile([128, 65], FP32, tag="bank_v", name="au")
            for s in range(4):
                h = g4 * 4 + s
                for i, (j0, jsz) in enumerate(NTS):
                    nc.tensor.matmul(
                        au[s * 32:s * 32 + 32, :],
                        PT4[i][:jsz, g4 * 128 + s * 32:g4 * 128 + s * 32 + 32],
                        v_sb[i][:jsz, h * 65:(h + 1) * 65],
                        start=(i == 0), stop=(i == 4),
                        tile_position=(0, s * 32))
            av4 = p_w.tile([128, 65], BF16, tag="av4", bufs=2, name="av4")
            rp = p_w.tile([128, 1], FP32, tag="avrec", bufs=2)
            nc.vector.reciprocal(rp[:, :], au[:, 64:65])
            nc.vector.tensor_scalar(av4[:, 0:64], au[:, 0:64],
                                    rp[:, :], None, op0=ALU.mult)
            nc.vector.memset(av4[:, 64:65], 1.0)
            for s in range(4):
                h = g4 * 4 + s
                t = p_w.tile([32, 65], BF16, tag=f"avh{h % 6}", bufs=2,
                             name=f"avh{h}")
                nc.sync.dma_start(t[:], av4[s * 32:s * 32 + 32, :])
                avh.append(t)

        if PH < 4:
            continue
        # ---- moba per g4: W^T, qk^T, EW, val, divide ----
        # outT2 tiles alias the dead BT slots (same tags)
        outT2 = [p_w.tile([128, N], BF16, tag=f"bt{g}", name=f"outT2_{g}")
                 for g in range(CTS)]
        ngroups = min(3, PH - 3)
        for g4 in range(ngroups):
            last = g4 == ngroups - 1
            if not last:
                den4 = p_w.tile([4, N], FP32, tag="den4", bufs=1, name="den4")
            numT_h = []
            for pp in range(2):  # head pairs (s=2pp, 2pp+1)
                if last:
                    den4 = p_w.tile([4, N], FP32, tag="den4", bufs=1,
                                    name="den4")
                ew_pair = []
                for i, (j0, jsz) in enumerate(NTS):
                    wt_ps, qm_ps = [], []
                    for hh in range(2):
                        s = 2 * pp + hh
                        b32 = s * 32
                        wa = ps_w.tile([128, 512], FP32, tag="bank_w")
                        nc.tensor.matmul(wa[:jsz, :],
                                         mask12[g4][b32:b32 + 32, j0:j0 + jsz],
                                         selT4[g4][b32:b32 + 32, 0:512],
                                         start=True, stop=True,
                                         tile_position=(b32, 0))
                        wt_ps.append(wa)
                    for hh in range(2):
                        s = 2 * pp + hh
                        b32 = s * 32
                        wb = ps_w.tile([128, 65], FP32, tag="bank_w")
                        nc.tensor.matmul(wb[:jsz, :],
                                         mask12[g4][b32:b32 + 32, j0:j0 + jsz],
                                         selT4[g4][b32:b32 + 32, 512:577],
                                         start=True, stop=True,
                                         tile_position=(b32, 0))
                        wt_ps.append(wb)
                    for hh in range(2):
                        s = 2 * pp + hh
                        h = g4 * 4 + s
                        rk = (h % 2) * 64
                        kt, qt = 6 + h // 2, h // 2
                        qa = ps_a.tile([128, 512], FP32, tag="bank_a")
                        nc.tensor.matmul(qa[:jsz, :],
                                         qkTbf[kt][rk:rk + 64, j0:j0 + jsz],
                                         qkTbf[qt][rk:rk + 64, 0:512],
                                         start=True, stop=True)
                        qm_ps.append(qa)
                    for hh in range(2):
                        s = 2 * pp + hh
                        h = g4 * 4 + s
                        rk = (h % 2) * 64
                        kt, qt = 6 + h // 2, h // 2
                        qb = ps_v.tile([128, 65], FP32, tag="bank_v")
                        nc.tensor.matmul(qb[:jsz, :],
                                         qkTbf[kt][rk:rk + 64, j0:j0 + jsz],
                                         qkTbf[qt][rk:rk + 64, 512:577],
                                         start=True, stop=True)
                        qm_ps.append(qb)
                    # e = exp (scalar), EW = e * W-psum (DVE)
                    for hh in range(2):
                        e_a = p_w.tile([128, 512], BF16, tag="exp_a", bufs=3)
                        e_b = p_w.tile([128, 65], BF16, tag="exp_b", bufs=3)
                        nc.scalar.activation(e_a[:jsz, :], qm_ps[hh][:jsz, :],
                                             ACTF.Exp, scale=SCALE)
                        nc.scalar.activation(e_b[:jsz, :],
                                             qm_ps[2 + hh][:jsz, :],
                                             ACTF.Exp, scale=SCALE)
                        t = p_ew.tile([128, N], BF16, tag="ew")
                        nc.vector.tensor_tensor(t[:jsz, 0:512], e_a[:jsz, :],
                                                wt_ps[hh][:jsz, :],
                                                op=ALU.mult)
                        nc.vector.tensor_tensor(t[:jsz, 512:577], e_b[:jsz, :],
                                                wt_ps[2 + hh][:jsz, :],
                                                op=ALU.mult)
                        ew_pair.append(t)
                # val per head of the pair
                for hh in range(2):
                    s = 2 * pp + hh
                    h = g4 * 4 + s
                    val_a = ps_v.tile([65, 512], FP32, tag="bank_v")
                    val_b = ps_v.tile([65, 65], FP32, tag="bank_v")
                    nc.tensor.matmul(val_a[:, :], avh[h][0:32, :],
                                     eah[h][0:32, 0:512],
                                     start=True, stop=False)
                    nc.tensor.matmul(val_b[:, :], avh[h][0:32, :],
                                     eah[h][0:32, 512:577],
                                     start=True, stop=False)
                    for i, (j0, jsz) in enumerate(NTS):
                        ewt = ew_pair[i * 2 + hh]
                        nc.tensor.matmul(val_a[:, :],
                                         v_sb[i][:jsz, h * 65:(h + 1) * 65],
                                         ewt[:jsz, 0:512],
                                         start=False, stop=(i == 4))
                        nc.tensor.matmul(val_b[:, :],
                                         v_sb[i][:jsz, h * 65:(h + 1) * 65],
                                         ewt[:jsz, 512:577],
                                         start=False, stop=(i == 4))
                    numT = p_w.tile([65, N], FP32, tag="numT", bufs=4,
                                    name="numT")
                    nc.scalar.copy(numT[:, 0:512], val_a[:])
                    nc.scalar.copy(numT[:, 512:577], val_b[:])
                    nc.sync.dma_start(den4[hh if last else s:
                                           (hh if last else s) + 1, :],
                                      numT[64:65, :])
                    numT_h.append(numT)
                if last:
                    # last group: per-pair reciprocal shortens the tail
                    # dependency chain into proj
                    nc.vector.reciprocal(den4[0:2, :], den4[0:2, :])
                    for hh in range(2):
                        s = 2 * pp + hh
                        _divide_head(nc, p_w, g4, s, den4, hh,
                                     numT_h[s], outT2)
            if not last:
                # batched reciprocal for the 4 heads, then divide + place
                nc.vector.reciprocal(den4[:, :], den4[:, :])
                for s in range(4):
                    _divide_head(nc, p_w, g4, s, den4, s, numT_h[s], outT2)

        if DBG and b == 0:
            nc.sync.dma_start(io["dbg_rak"], rak4[0][:, :])
            nc.sync.dma_start(io["dbg_pt"], PT4[0][:, :])
            nc.sync.dma_start(io["dbg_qkb"], qkTbf[0][:, :])
            nc.sync.dma_start(io["dbg_av"],
                              [t for t in (avh[0],)][0][:, :])
            nc.sync.dma_start(io["dbg_sel"], selT4[0][:, :])
            nc.sync.dma_start(io["dbg_eah"], eah[0][:, :])
            nc.sync.dma_start(io["dbg_out2"], outT2[0][:, :])
            nc.sync.dma_start(io["dbg_mk"], mask12[0][:, :])

        # ---- proj: head-pair K=128 matmuls + bias ----
        if PH < 9:
            continue
        for i, (n0, nsz) in enumerate(NTS):
            pr_a = ps_a.tile([128, 512], FP32, tag="bank_a")
            pr_b = ps_a.tile([128, 256], FP32, tag="bank_a")
            for g in range(CTS):
                nc.tensor.matmul(pr_a[:nsz, :], outT2[g][:, n0:n0 + nsz],
                                 wproj2[g][:, 0:512],
                                 start=(g == 0), stop=(g == 5))
                nc.tensor.matmul(pr_b[:nsz, :], outT2[g][:, n0:n0 + nsz],
                                 wproj2[g][:, 512:768],
                                 start=(g == 0), stop=(g == 5))
            o_sb = p_w.tile([128, C], FP32, tag="wqkt", bufs=2, name="o_sb")
            nc.vector.tensor_tensor(o_sb[:nsz, 0:512], pr_a[:nsz, :],
                                    bb128[:nsz, 0:512], op=ALU.add)
            nc.vector.tensor_tensor(o_sb[:nsz, 512:768], pr_b[:nsz, :],
                                    bb128[:nsz, 512:768], op=ALU.add)
            nc.sync.dma_start(io["out"][b, n0:n0 + nsz, :], o_sb[:nsz, :])


_PROG = None


def _build_program():
    global _PROG
    if _PROG is not None:
        return _PROG
    nc = bacc.Bacc("TRN2", target_bir_lowering=False, debug=False)
    io = {
        "xT_f32": nc.dram_tensor("xT_f32", [NB, C, N], FP32,
                                 kind="ExternalInput").ap(),
        "xT_bf16": nc.dram_tensor("xT_bf16", [NB, C, N], BF16,
                                  kind="ExternalInput").ap(),
        "wq32": nc.dram_tensor("wq32", [C, C], FP32,
                               kind="ExternalInput").ap(),
        "wqkT": nc.dram_tensor("wqkT", [2 * C, C], FP32,
                               kind="ExternalInput").ap(),
        "wqk_bf": nc.dram_tensor("wqk_bf", [C, 2 * C], BF16,
                                 kind="ExternalInput").ap(),
        "wv": nc.dram_tensor("wv", [C, C], BF16, kind="ExternalInput").ap(),
        "wproj": nc.dram_tensor("wproj", [C, C], BF16,
                                kind="ExternalInput").ap(),
        "bproj": nc.dram_tensor("bproj", [1, C], BF16,
                                kind="ExternalInput").ap(),
        "out": nc.dram_tensor("out", [NB, N, C], FP32,
                              kind="ExternalOutput").ap(),
    }
    if DBG:
        io["dbg_rak"] = nc.dram_tensor("dbg_rak", [128, N], FP32,
                                       kind="ExternalOutput").ap()
        io["dbg_pt"] = nc.dram_tensor("dbg_pt", [128, 384], BF16,
                                      kind="ExternalOutput").ap()
        io["dbg_qkb"] = nc.dram_tensor("dbg_qkb", [128, N], BF16,
                                       kind="ExternalOutput").ap()
        io["dbg_av"] = nc.dram_tensor("dbg_av", [32, 65], BF16,
                                      kind="ExternalOutput").ap()
        io["dbg_sel"] = nc.dram_tensor("dbg_sel", [128, N], BF16,
                                       kind="ExternalOutput").ap()
        io["dbg_eah"] = nc.dram_tensor("dbg_eah", [32, N], BF16,
                                       kind="ExternalOutput").ap()
        io["dbg_out2"] = nc.dram_tensor("dbg_out2", [128, N], BF16,
                                        kind="ExternalOutput").ap()
        io["dbg_mk"] = nc.dram_tensor("dbg_mk", [128, N], BF16,
                                      kind="ExternalOutput").ap()
    with tile.TileContext(nc) as tc:
        with ExitStack() as stack:
            tc._ctx = stack
            _emit(tc, io)
    nc.compile()
    _PROG = (nc, io)
    return _PROG


def make_in_maps(x, Wqkv, Wproj, bproj):
    """Shard full inputs into per-core input maps."""
    bf16 = ml_dtypes.bfloat16
    x = np.ascontiguousarray(x, np.float32)
    Wqkv = np.asarray(Wqkv, np.float32)
    wq32 = np.ascontiguousarray(Wqkv[:, :C])
    wqkT = np.ascontiguousarray(Wqkv[:, :2 * C].T)
    wqk_bf = np.ascontiguousarray(Wqkv[:, :2 * C]).astype(bf16)
    wv = np.ascontiguousarray(Wqkv[:, 2 * C:]).astype(bf16)
    wproj = np.ascontiguousarray(Wproj, np.float32).astype(bf16)
    bp = np.asarray(bproj, np.float32).reshape(1, C).astype(bf16)
    in_maps = []
    for core in range(NCORES):
        xs = x[core * NB:(core + 1) * NB]  # [2, N, C]
        xT = np.ascontiguousarray(xs.transpose(0, 2, 1))  # [2, C, N]
        in_maps.append({
            "xT_f32": xT,
            "xT_bf16": xT.astype(bf16),
            "wq32": wq32,
            "wqkT": wqkT,
            "wqk_bf": wqk_bf,
            "wv": wv,
            "wproj": wproj,
            "bproj": bp,
        })
    return in_maps


def kernel(x, Wqkv, Wproj, bproj):
    nc, _ = _build_program()
    in_maps = make_in_maps(x, Wqkv, Wproj, bproj)
    res = run_bass_kernel_spmd(nc, in_maps, list(range(NCORES)))
    outs = [r["out"] for r in res.results]
    return np.concatenate(outs, axis=0).astype(np.float32)


if __name__ == "__main__":
    _build_program()
    print("BUILD OK")
